# revision 3
# baseline (speedup 1.0000x reference)
"""Trainium2 Bass kernel for nn_EnhancedTransformerModel (B=4,S=256,E=512,H=8,F=2048,L=4,V=32000).

Sharding (8 cores):
  - Encoder token-split: core c handles batch c//2, token half c%2 (128 tokens),
    all 8 heads. K^T/V exchanged within each 2-core batch pair via AllGather.
  - Rel-pos bias: P = q @ T_win^T per head; the per-row diagonal shift (skew) is
    realized by a DRAM round trip (write [128,384] rows, stride-382 read back)
    accumulated onto the logits with a DMA-accumulate. Per-core T windows absorb
    the l-tile offset so the program is identical on all cores (SPMD).
  - Decoder: final hidden states AllGathered 8-way (transposed); each core
    computes all 1024 tokens x a 4000-wide vocab slice.

Dtypes: bf16 matmuls with fp32 PSUM accumulation; fp32 residual stream + LN.
"""

import sys

if "/opt/trn_rl_repo" not in sys.path:
    sys.path.insert(0, "/opt/trn_rl_repo")

import math
import numpy as np
import ml_dtypes

import concourse.bass as bass
import concourse.bacc as bacc
import concourse.mybir as mybir
import concourse.tile as tile
from concourse.masks import make_identity
from concourse.bass_utils import run_bass_kernel_spmd

DT = mybir.dt
AF = mybir.ActivationFunctionType
ALU = mybir.AluOpType

B, S, E, H, F, L, V = 4, 256, 512, 8, 2048, 4, 32000
HD = E // H  # 64
N_CORES = 8
VS = V // N_CORES    # vocab slice per core = 4000
NV = 8               # vocab N-tiles per core
VN = VS // NV        # 500
P = 128
ET = E // P          # 4 e-tiles
FT = F // P          # 16 f-tiles
TW = 384             # padded T window width (383 used)
RNK = ET * P * P + P * E   # bf16 elements per rank in the kv exchange

bf16 = ml_dtypes.bfloat16

_CACHE = {}


def build_nc():
    nc = bacc.Bacc(target_bir_lowering=False, num_devices=N_CORES)

    # ---------------- DRAM I/O ----------------
    x0 = nc.dram_tensor("x0", [P, E], DT.float32, kind="ExternalInput")
    mask_in = nc.dram_tensor("mask_in", [P, S], DT.bfloat16, kind="ExternalInput")
    twin = nc.dram_tensor("twin", [HD, H, TW], DT.bfloat16, kind="ExternalInput")
    wqk = [nc.dram_tensor(f"wqk{l}", [P, ET, 8, P], DT.bfloat16, kind="ExternalInput") for l in range(L)]
    wv = [nc.dram_tensor(f"wv{l}", [P, ET, E], DT.bfloat16, kind="ExternalInput") for l in range(L)]
    wo = [nc.dram_tensor(f"wo{l}", [P, ET, E], DT.bfloat16, kind="ExternalInput") for l in range(L)]
    w1 = [nc.dram_tensor(f"w1{l}", [P, ET, FT, P], DT.bfloat16, kind="ExternalInput") for l in range(L)]
    w2 = [nc.dram_tensor(f"w2{l}", [P, FT, E], DT.bfloat16, kind="ExternalInput") for l in range(L)]
    dw = nc.dram_tensor("dw", [P, ET, VS], DT.bfloat16, kind="ExternalInput")

    out_logits = nc.dram_tensor("out_logits", [N_CORES, P, VS], DT.float32, kind="ExternalOutput")
    dbg = nc.dram_tensor("dbg", [L + 2, P, E], DT.float32, kind="ExternalOutput")

    rg_pair = [[0, 1], [2, 3], [4, 5], [6, 7]]
    rg_all = [list(range(N_CORES))]

    with tile.TileContext(nc) as tc:
        with (
            tc.tile_pool(name="const", bufs=1) as constp,
            tc.tile_pool(name="resid", bufs=1) as residp,
            tc.tile_pool(name="wpool", bufs=2) as wpool,
            tc.tile_pool(name="w1pool", bufs=1) as w1pool,
            tc.tile_pool(name="w2pool", bufs=1) as w2pool,
            tc.tile_pool(name="dwpool", bufs=1) as dwpool,
            tc.tile_pool(name="act", bufs=2) as actp,
            tc.tile_pool(name="attn", bufs=3) as attnp,
            tc.tile_pool(name="small", bufs=4) as smallp,
            tc.tile_pool(name="outp", bufs=4) as outp,
            tc.tile_pool(name="ps", bufs=2, space="PSUM") as psp,
            tc.tile_pool(name="dram", bufs=3, space="DRAM") as dramp,
        ):
            # ---------------- constants ----------------
            ident = constp.tile([P, P], DT.bfloat16)
            make_identity(nc, ident[:])
            mask_t = constp.tile([P, S], DT.bfloat16)
            nc.sync.dma_start(mask_t[:], mask_in[:])
            twin_t = constp.tile([HD, H, TW], DT.bfloat16)
            nc.sync.dma_start(twin_t[:], twin[:])
            eps_t = constp.tile([P, 1], DT.float32)
            nc.gpsimd.memset(eps_t[:], 1e-5)
            dw_t = dwpool.tile([P, ET, VS], DT.bfloat16)
            nc.sync.dma_start(dw_t[:], dw[:])

            # residual stream (fp32)
            x = residp.tile([P, E], DT.float32)
            nc.sync.dma_start(x[:], x0[:])

            # ---------------- helpers ----------------
            def layer_norm(dst, src):
                """dst = (src - mean)/sqrt(var+eps); src fp32 [P, E]."""
                stats = smallp.tile([P, 6], DT.float32, tag="ln_stats", name="stats")
                mv = smallp.tile([P, 2], DT.float32, tag="ln_mv", name="mv")
                nc.vector.bn_stats(out=stats[:], in_=src[:])
                nc.vector.bn_aggr(out=mv[:], in_=stats[:])
                rstd = smallp.tile([P, 1], DT.float32, tag="ln_rstd", name="rstd")
                nc.scalar.activation(out=rstd[:], in_=mv[:, 1:2], func=AF.Sqrt,
                                     bias=eps_t[:], scale=1.0)
                nc.vector.reciprocal(out=rstd[:], in_=rstd[:])
                nc.vector.tensor_scalar(
                    out=dst[:], in0=src[:], scalar1=mv[:, 0:1], scalar2=rstd[:],
                    op0=ALU.subtract, op1=ALU.mult,
                )

            def transpose_to(dst3, src_bf, n_tiles):
                """dst3 [P, n_tiles, P] = per-tile transpose of src_bf [P, n_tiles*P]."""
                for t in range(n_tiles):
                    ptr = psp.tile([P, P], DT.bfloat16, tag="mm1", bufs=2, name="ptr")
                    nc.tensor.transpose(ptr[:], src_bf[:, t * P:(t + 1) * P], ident[:])
                    nc.scalar.activation(out=dst3[:, t, :], in_=ptr[:], func=AF.Copy)

            # ---------------- embedding LN ----------------
            xl = actp.tile([P, E], DT.float32, tag="xln")
            layer_norm(xl, x)
            nc.vector.tensor_copy(x[:], xl[:])
            nc.sync.dma_start(dbg[0], x[:])

            # ---------------- encoder layers ----------------
            for l in range(L):
                wqk_t = wpool.tile([P, ET, 8, P], DT.bfloat16, tag="wqk", name="wqk_t")
                nc.sync.dma_start(wqk_t[:], wqk[l][:])
                wv_t = wpool.tile([P, ET, E], DT.bfloat16, tag="wv", name="wv_t")
                nc.sync.dma_start(wv_t[:], wv[l][:])
                wo_t = wpool.tile([P, ET, E], DT.bfloat16, tag="wo", name="wo_t")
                nc.sync.dma_start(wo_t[:], wo[l][:])
                w1_t = w1pool.tile([P, ET, FT, P], DT.bfloat16, tag="w1", name="w1_t")
                nc.sync.dma_start(w1_t[:], w1[l][:])
                w2_t = w2pool.tile([P, FT, E], DT.bfloat16, tag="w2", name="w2_t")
                nc.sync.dma_start(w2_t[:], w2[l][:])

                # LN1 -> h (bf16) -> hT
                h_bf = actp.tile([P, E], DT.bfloat16, tag="h_bf", name="h_bf")
                layer_norm(h_bf, x)
                hT = actp.tile([P, ET, P], DT.bfloat16, tag="hT", name="hT")
                transpose_to(hT, h_bf, ET)

                # QKV
                qT = actp.tile([P, ET, P], DT.bfloat16, tag="qT", name="qT")
                kT = actp.tile([P, ET, P], DT.bfloat16, tag="kT", name="kT")
                for mt in range(8):
                    pq = psp.tile([P, P], DT.float32, tag="mm1", bufs=2, name="pq")
                    for et in range(ET):
                        nc.tensor.matmul(pq[:], wqk_t[:, et, mt, :], hT[:, et, :],
                                         start=(et == 0), stop=(et == ET - 1))
                    if mt < 4:
                        nc.vector.tensor_copy(qT[:, mt, :], pq[:])
                    else:
                        nc.vector.tensor_scalar_mul(kT[:, mt - 4, :], pq[:],
                                                    1.0 / math.sqrt(HD))
                pv = psp.tile([P, E], DT.float32, tag="big", bufs=2, name="pv")
                for et in range(ET):
                    nc.tensor.matmul(pv[:], hT[:, et, :], wv_t[:, et, :],
                                     start=(et == 0), stop=(et == ET - 1))
                v_mine = actp.tile([P, E], DT.bfloat16, tag="v_mine", name="v_mine")
                nc.scalar.activation(out=v_mine[:], in_=pv[:], func=AF.Copy)

                # local q roundtrip (per-head base-0 stationaries)
                qdram = dramp.tile([ET, P, P], DT.bfloat16, tag="qdram", name="qdram")
                nc.sync.dma_start(qdram[:].transpose([1, 0, 2]), qT[:])

                # kv allgather within batch pair
                kv_in = dramp.tile([RNK], DT.bfloat16, tag="kv_in", name="kv_in")
                nc.sync.dma_start(
                    kv_in[0:ET * P * P].rearrange("(e p t) -> p e t", e=ET, p=P),
                    kT[:])
                nc.sync.dma_start(
                    kv_in[ET * P * P:].rearrange("(p e) -> p e", p=P), v_mine[:])
                kv_out = dramp.tile([2 * RNK], DT.bfloat16, tag="kv_out", name="kv_out")
                nc.gpsimd.collective_compute(
                    "AllGather", ALU.bypass, replica_groups=rg_pair,
                    ins=[kv_in[:]], outs=[kv_out[:]],
                )
                kv2 = kv_out[:].rearrange("(r x) -> r x", r=2)
                # kfull: [HD, H, 2, TOK]; head h = rows h*64..h*64+63 of kT
                kfull = attnp.tile([HD, H, 2, P], DT.bfloat16, tag="kfull", bufs=2, name="kfull")
                for hh in range(H):
                    base = (hh // 2) * P * P + (hh % 2) * HD * P
                    src = kv2[:, base:base + HD * P].rearrange("r (d t) -> d r t", d=HD)
                    nc.sync.dma_start(kfull[:, hh, :, :], src)
                vfull = attnp.tile([P, 2, E], DT.bfloat16, tag="vfull", bufs=2, name="vfull")
                for r in range(2):
                    src = kv2[r, ET * P * P:].rearrange("(p e) -> p e", p=P)
                    nc.sync.dma_start(vfull[:, r, :], src)

                # attention per head
                oT = actp.tile([P, ET, P], DT.bfloat16, tag="oT", name="oT")
                ps_o = None
                for hh in range(H):
                    qs = smallp.tile([HD, P], DT.bfloat16, tag="qs", name="qs")
                    nc.sync.dma_start(
                        qs[:], qdram[hh // 2, (hh % 2) * HD:(hh % 2) * HD + HD, :])

                    ps_sc = psp.tile([P, S], DT.float32, tag="sc", bufs=2, name="ps_sc")
                    nc.tensor.matmul(ps_sc[:], qs[:], kfull[:, hh, :, :],
                                     start=True, stop=True)
                    ps_b = psp.tile([P, TW], DT.float32, tag="sc", bufs=2, name="ps_b")
                    nc.tensor.matmul(ps_b[:], qs[:], twin_t[:, hh, :],
                                     start=True, stop=True)
                    pt = attnp.tile([P, TW], DT.bfloat16, tag="pt", name="pt")
                    nc.scalar.activation(out=pt[:], in_=ps_b[:], func=AF.Copy)
                    pdram = dramp.tile([P * TW], DT.bfloat16, tag="pdram", name="pdram")
                    nc.sync.dma_start(pdram[:].rearrange("(p t) -> p t", p=P), pt[:])

                    logits = attnp.tile([P, S], DT.bfloat16, tag="logits", name="logits")
                    nc.vector.tensor_tensor(logits[:], ps_sc[:], mask_t[:], ALU.add)
                    # skew accumulate: logits[r, m] += pdram[r*384 + m - r + 127]
                    skew = bass.AP(pdram.tensor, pdram.offset + 127,
                                   [[TW - 2, P], [1, S]])
                    nc.gpsimd.dma_start(logits[:], skew, accum_op=ALU.add)
                    attn_e = attnp.tile([P, S], DT.bfloat16, tag="attn_e", name="attn_e")
                    nc.scalar.activation(out=attn_e[:], in_=logits[:], func=AF.Exp)
                    zsum = smallp.tile([P, 1], DT.float32, tag="zsum", name="zsum")
                    nc.vector.reduce_sum(out=zsum[:], in_=attn_e[:],
                                         axis=mybir.AxisListType.X)
                    rz = smallp.tile([P, 1], DT.float32, tag="rz", name="rz")
                    nc.vector.reciprocal(out=rz[:], in_=zsum[:])
                    attn_n = attnp.tile([P, S], DT.bfloat16, tag="attn_n", name="attn_n")
                    nc.vector.tensor_scalar_mul(attn_n[:], attn_e[:], rz[:])

                    attnT = attnp.tile([P, 2, P], DT.bfloat16, tag="attnT", name="attnT")
                    for mt in range(2):
                        ptr2 = psp.tile([P, P], DT.bfloat16, tag="mm1", bufs=2, name="ptr2")
                        nc.tensor.transpose(ptr2[:], attn_n[:, mt * P:(mt + 1) * P],
                                            ident[:])
                        nc.scalar.activation(out=attnT[:, mt, :], in_=ptr2[:],
                                             func=AF.Copy)

                    if hh % 2 == 0:
                        ps_o = psp.tile([P, P], DT.float32, tag="ps_o", bufs=2, name="ps_o")
                    r0 = (hh % 2) * HD
                    for mt in range(2):
                        nc.tensor.matmul(
                            ps_o[r0:r0 + HD, :],
                            vfull[:, mt, hh * HD:(hh + 1) * HD], attnT[:, mt, :],
                            start=(mt == 0), stop=(mt == 1),
                            tile_position=(0, r0),
                        )
                    if hh % 2 == 1:
                        nc.vector.tensor_copy(oT[:, hh // 2, :], ps_o[:])

                # out-proj + residual
                px = psp.tile([P, E], DT.float32, tag="big", bufs=2, name="px")
                for kt in range(ET):
                    nc.tensor.matmul(px[:], oT[:, kt, :], wo_t[:, kt, :],
                                     start=(kt == 0), stop=(kt == ET - 1))
                nc.vector.tensor_tensor(x[:], px[:], x[:], ALU.add)

                # FFN
                h2 = actp.tile([P, E], DT.bfloat16, tag="h_bf", name="h2")
                layer_norm(h2, x)
                h2T = actp.tile([P, ET, P], DT.bfloat16, tag="hT", name="h2T")
                transpose_to(h2T, h2, ET)
                fT = actp.tile([P, FT, P], DT.bfloat16, tag="fT", name="fT")
                for ft in range(FT):
                    pf = psp.tile([P, P], DT.float32, tag="mm1", bufs=2, name="pf")
                    for et in range(ET):
                        nc.tensor.matmul(pf[:], w1_t[:, et, ft, :], h2T[:, et, :],
                                         start=(et == 0), stop=(et == ET - 1))
                    nc.scalar.activation(out=fT[:, ft, :], in_=pf[:], func=AF.Gelu)
                px2 = psp.tile([P, E], DT.float32, tag="big", bufs=2, name="px2")
                for ft in range(FT):
                    nc.tensor.matmul(px2[:], fT[:, ft, :], w2_t[:, ft, :],
                                     start=(ft == 0), stop=(ft == FT - 1))
                nc.vector.tensor_tensor(x[:], px2[:], x[:], ALU.add)
                nc.sync.dma_start(dbg[1 + l], x[:])

            # ---------------- final LN + 8-way allgather ----------------
            xf = actp.tile([P, E], DT.float32, tag="xln", name="xf")
            layer_norm(xf, x)
            nc.sync.dma_start(dbg[L + 1], xf[:])
            xf_bf = actp.tile([P, E], DT.bfloat16, tag="h_bf", name="xf_bf")
            nc.vector.tensor_copy(xf_bf[:], xf[:])
            xfT = actp.tile([P, ET, P], DT.bfloat16, tag="hT", name="xfT")
            transpose_to(xfT, xf_bf, ET)
            xf_in = dramp.tile([ET * P * P], DT.bfloat16, tag="xf_in", name="xf_in")
            nc.sync.dma_start(
                xf_in[:].rearrange("(e p t) -> p e t", e=ET, p=P), xfT[:])
            xf_out = dramp.tile([N_CORES * ET * P * P], DT.bfloat16, tag="xf_out",
                                name="xf_out", addr_space="Shared")
            nc.gpsimd.collective_compute(
                "AllGather", ALU.bypass, replica_groups=rg_all,
                ins=[xf_in[:]], outs=[xf_out[:]],
            )
            xfT_all = dwpool.tile([P, N_CORES * ET, P], DT.bfloat16, name="xfT_all")
            nc.sync.dma_start(
                xfT_all[:],
                xf_out[:].rearrange("(g p t) -> p g t", g=N_CORES * ET, p=P))

            # ---------------- decoder ----------------
            for tt in range(N_CORES):
                for nt in range(NV):
                    pd = psp.tile([P, VN], DT.float32, tag="big", bufs=2, name="pd")
                    for et in range(ET):
                        nc.tensor.matmul(
                            pd[:], xfT_all[:, tt * ET + et, :],
                            dw_t[:, et, nt * VN:(nt + 1) * VN],
                            start=(et == 0), stop=(et == ET - 1))
                    ot = outp.tile([P, VN], DT.float32, tag="ot", name="ot")
                    if nt % 2 == 0:
                        nc.vector.tensor_copy(ot[:], pd[:])
                    else:
                        nc.scalar.activation(out=ot[:], in_=pd[:], func=AF.Copy)
                    nc.sync.dma_start(out_logits[tt, :, nt * VN:(nt + 1) * VN], ot[:])

    nc.compile()
    return nc


def host_prep(inputs):
    """Build the 8 per-core input maps."""
    src = np.asarray(inputs["src"])
    emb = np.asarray(inputs["emb"], np.float32)
    rel_table = np.asarray(inputs["rel_table"], np.float32)
    inW = np.asarray(inputs["inW"], np.float32)
    outW = np.asarray(inputs["outW"], np.float32)
    w1 = np.asarray(inputs["w1"], np.float32)
    w2 = np.asarray(inputs["w2"], np.float32)
    dec_w = np.asarray(inputs["dec_w"], np.float32)

    # structurally-fixed params: verify assumptions
    for name in ("norm_in_b", "inB", "outB", "ln1_b", "ln2_b", "b1", "b2",
                 "normf_b", "dec_b"):
        assert np.abs(np.asarray(inputs[name])).max() == 0.0, name
    for name in ("norm_in_s", "ln1_s", "ln2_s", "normf_s"):
        a = np.asarray(inputs[name])
        assert np.abs(a - 1.0).max() == 0.0, name

    x_emb = emb[src].astype(np.float32) * math.sqrt(E)  # [B, S, E]

    per_layer = []
    for l in range(L):
        wqk_l = np.ascontiguousarray(
            inW[l][:1024].reshape(8, P, ET, P).transpose(3, 2, 0, 1)).astype(bf16)
        wv_l = np.ascontiguousarray(
            inW[l][1024:].reshape(E, ET, P).transpose(2, 1, 0)).astype(bf16)
        wo_l = np.ascontiguousarray(
            outW[l].T.reshape(ET, P, E).transpose(1, 0, 2)).astype(bf16)
        w1_l = np.ascontiguousarray(
            w1[l].reshape(FT, P, ET, P).transpose(3, 2, 0, 1)).astype(bf16)
        w2_l = np.ascontiguousarray(
            w2[l].T.reshape(FT, P, E).transpose(1, 0, 2)).astype(bf16)
        per_layer.append((wqk_l, wv_l, wo_l, w1_l, w2_l))

    in_maps = []
    for c in range(N_CORES):
        b = c // 2
        L0 = (c % 2) * P
        m = {}
        m["x0"] = np.ascontiguousarray(x_emb[b, L0:L0 + P])
        rows = np.arange(L0, L0 + P)
        mask = (np.arange(S)[None, :] > rows[:, None]).astype(np.float32)
        m["mask_in"] = mask.astype(bf16)
        # twin[d, h, jj] = rel_table[jj + 128 - L0, h*64 + d], jj in [0, 384)
        jidx = np.arange(TW) + 128 - L0
        tw = rel_table[jidx]  # [384, 512]
        m["twin"] = np.ascontiguousarray(
            tw.reshape(TW, H, HD).transpose(2, 1, 0)).astype(bf16)
        for l in range(L):
            wqk_l, wv_l, wo_l, w1_l, w2_l = per_layer[l]
            m[f"wqk{l}"] = wqk_l
            m[f"wv{l}"] = wv_l
            m[f"wo{l}"] = wo_l
            m[f"w1{l}"] = w1_l
            m[f"w2{l}"] = w2_l
        VOFF = c * VS
        m["dw"] = np.ascontiguousarray(
            dec_w[VOFF:VOFF + VS].T.reshape(ET, P, VS).transpose(1, 0, 2)).astype(bf16)
        in_maps.append(m)
    return in_maps


def assemble(results):
    """results[c]["out_logits"] [8, 128, 4000] -> [B, S, V] fp32."""
    out = np.empty((B, S, V), np.float32)
    for c in range(N_CORES):
        VOFF = c * VS
        lg = results[c]["out_logits"]  # [8, 128, VS]
        for tt in range(N_CORES):
            b = tt // 2
            s0 = (tt % 2) * P
            out[b, s0:s0 + P, VOFF:VOFF + VS] = lg[tt]
    return out


def get_nc():
    if "nc" not in _CACHE:
        _CACHE["nc"] = build_nc()
    return _CACHE["nc"]


def kernel(**inputs):
    nc = get_nc()
    in_maps = host_prep(inputs)
    res = run_bass_kernel_spmd(nc, in_maps, list(range(N_CORES)))
    _CACHE["last_results"] = res.results
    return assemble(res.results)


if __name__ == "__main__":
    import reference

    inputs = {k: np.asarray(v) for k, v in reference.setup_inputs().items()}
    out = kernel(**inputs)
    exp = np.asarray(reference.reference(**inputs))
    err = np.abs(out - exp).max()
    print("abs err:", err, "rel:", err / np.abs(exp).max())
